# revision 8
# baseline (speedup 1.0000x reference)
"""Trainium2 Bass kernel for nn_ConvPoolBlock (GCN x2 + EdgePooling).

Strategy (8 NeuronCores, SPMD single NEFF):
  - Nodes are sharded contiguously across cores (6250/core, padded to 6272).
  - Edges (incl. GCN self-loops) are assigned to the core owning their dst
    node, sorted by dst, grouped into 64-node windows, padded to 128-edge
    tiles.
  - Per layer: indirect-DMA gather of source-node rows, per-tile one-hot
    selection matrix (built in one DVE op), PE matmul accumulates the
    segment-sum in PSUM per window, then a folded (BN+GCN-norm+bias) weight
    matmul + ReLU produces the layer output window.  BN scale/shift, degree
    normalization and biases are folded into the weights / rank-1 terms on
    the host (structure-only preprocessing).
  - One AllGather between the two conv layers redistributes full x1.
  - The final pool BN is folded into per-node score projections p, q which
    the device computes per window.
  - Host: edge scores e = p[src]+q[dst]+k0, softmax grouped by dst, the
    (inherently sequential) greedy edge matching via the exact
    locally-dominant-edge iteration, cluster building, and the final
    new_x/new_edge_index/new_batch assembly.
"""
import math
import os

import numpy as np

# Problem constants (hardcoded per spec).
N, E, C, NGRAPH = 50000, 250000, 128, 64
LAYERS = 2
EPS = 1e-5
ADD_TO_EDGE_SCORE = 0.5

NCORES = 8
NPC = N // NCORES          # real nodes per core (6250)
WIN = 64                   # nodes per aggregation window
LOCAL = 6272               # padded nodes per core (multiple of 128)
NW = LOCAL // WIN          # windows per core (98)
NPAIR = NW // 2            # window pairs (128-node groups) per core (49)
AUGN = LOCAL * NCORES      # augmented global node count (50176)
TILE = 128                 # edges per tile
KT = 16                    # tiles per gather chunk

_DUMMY = NPC               # augmented-local id of a guaranteed zero row (pad)


def _aug_id(node):
    """Original node id -> augmented (padded) id."""
    c = node // NPC
    return c * LOCAL + (node - c * NPC)


def _prep_structure(edge_index):
    """Host-side graph preprocessing (int/scalar structure only)."""
    src = np.asarray(edge_index[0]).astype(np.int64)
    dst = np.asarray(edge_index[1]).astype(np.int64)
    loop = np.arange(N, dtype=np.int64)
    all_src = np.concatenate([src, loop])
    all_dst = np.concatenate([dst, loop])

    deg = np.bincount(all_dst, minlength=N).astype(np.float64)
    dinv64 = 1.0 / np.sqrt(deg)
    dinv = dinv64.astype(np.float32)
    w_all = (dinv[all_src] * dinv[all_dst]).astype(np.float32)

    # t0[d] = sum of edge weights (incl self loop) into d
    t0 = np.bincount(all_dst, weights=w_all.astype(np.float64), minlength=N)
    t0 = t0.astype(np.float32)

    core = all_dst // NPC
    local = all_dst - core * NPC
    win = local // WIN
    dstloc = (local - win * WIN).astype(np.float32)
    key = (core * NW + win).astype(np.int64)

    order = np.argsort(key, kind="stable")
    key_s = key[order]
    src_s = all_src[order]
    dstloc_s = dstloc[order]
    w_s = w_all[order]

    ngroups = NCORES * NW
    cnt = np.bincount(key_s, minlength=ngroups)          # [ngroups]
    grp_start = np.concatenate([[0], np.cumsum(cnt)[:-1]])
    rank = np.arange(len(key_s)) - grp_start[key_s]      # rank within group

    cnt2 = cnt.reshape(NCORES, NW)
    tiles_cw = -(-cnt2 // TILE)                          # ceil
    tw = tiles_cw.max(axis=0)                            # [NW] max over cores
    tw = np.maximum(tw, 1)
    woff = np.concatenate([[0], np.cumsum(tw)[:-1]])     # tile offset of window
    T = int(tw.sum())

    # padded per-core tile arrays
    esrc = np.full((NCORES, T * TILE), _DUMMY, np.int32)
    edst = np.full((NCORES, T * TILE), -1.0, np.float32)
    ew = np.zeros((NCORES, T * TILE), np.float32)

    cores_s = key_s // NW
    wins_s = key_s - cores_s * NW
    pos = woff[wins_s] * TILE + rank                     # position in stream
    src_aug = (src_s // NPC) * LOCAL + (src_s % NPC)
    esrc[cores_s, pos] = src_aug.astype(np.int32)
    edst[cores_s, pos] = dstloc_s
    ew[cores_s, pos] = w_s

    # transpose to [NCORES, 128, T] (partition-major for SBUF)
    esrcT = np.ascontiguousarray(esrc.reshape(NCORES, T, TILE).transpose(0, 2, 1))
    edstT = np.ascontiguousarray(edst.reshape(NCORES, T, TILE).transpose(0, 2, 1))
    ewT = np.ascontiguousarray(ew.reshape(NCORES, T, TILE).transpose(0, 2, 1))

    # t01 [NCORES, 2, LOCAL]: row0 = t0 (pads 0), row1 = ones
    t01 = np.zeros((NCORES, 2, LOCAL), np.float32)
    t01[:, 1, :] = 1.0
    t01[:, 0, :NPC] = t0.reshape(NCORES, NPC)

    return dict(
        T=T, tw=tw, woff=woff, esrcT=esrcT, edstT=edstT, ewT=ewT,
        t01=t01, dinv=dinv, src=src, dst=dst,
    )


def _fold_weights(inp):
    """Fold BN/bias/pool params into matmul weights (host, f32)."""
    out = {}
    wmats = np.zeros((LAYERS, C, C), np.float32)
    rank1 = np.zeros((LAYERS, 2, C), np.float32)
    for i in range(LAYERS):
        a = (inp["bn_gamma"][i] / np.sqrt(inp["bn_var"][i] + EPS)).astype(np.float32)
        c = (inp["bn_beta"][i] - inp["bn_mean"][i] * a).astype(np.float32)
        wmats[i] = a[:, None] * inp["conv_w"][i]
        rank1[i, 0] = c @ inp["conv_w"][i]
        rank1[i, 1] = inp["conv_b"][i]
    pa = (inp["pool_bn_gamma"] / np.sqrt(inp["pool_bn_var"] + EPS)).astype(np.float32)
    pc = (inp["pool_bn_beta"] - inp["pool_bn_mean"] * pa).astype(np.float32)
    w1 = np.asarray(inp["pool_w"][:C])
    w2 = np.asarray(inp["pool_w"][C:])
    out["wmats"] = wmats
    out["rank1"] = rank1
    out["w1rep"] = np.broadcast_to(pa * w1, (128, C)).copy().astype(np.float32)
    out["w2rep"] = np.broadcast_to(pa * w2, (128, C)).copy().astype(np.float32)
    out["k0"] = float(pc @ (w1 + w2) + np.asarray(inp["pool_b"]))
    out["pa"] = pa
    out["pc"] = pc
    return out


def _build_bass(T, tw, woff):
    """Build the SPMD Bass/Tile program (shared by all 8 cores)."""
    import concourse.bass as bass
    import concourse.bacc as bacc
    import concourse.mybir as mybir
    from concourse.tile import TileContext

    f32 = mybir.dt.float32
    i32 = mybir.dt.int32

    nc = bacc.Bacc()
    x_aug = nc.declare_dram_parameter("x_aug", [AUGN, C], f32, isOutput=False)
    esrcT_d = nc.declare_dram_parameter("esrcT", [128, T], i32, isOutput=False)
    econst_d = nc.declare_dram_parameter("econst", [128, 2 * T + WIN], f32,
                                         isOutput=False)
    t01_d = nc.declare_dram_parameter("t01", [2, LOCAL], f32, isOutput=False)
    wmats_d = nc.declare_dram_parameter("wmats", [LAYERS, C, C], f32, isOutput=False)
    rank1_d = nc.declare_dram_parameter("rank1", [LAYERS, 2, C], f32, isOutput=False)
    w1rep_d = nc.declare_dram_parameter("w1rep", [128, C], f32, isOutput=False)
    w2rep_d = nc.declare_dram_parameter("w2rep", [128, C], f32, isOutput=False)

    x2_out = nc.declare_dram_parameter("x2_out", [LOCAL, C], f32, isOutput=True)
    pq_out = nc.declare_dram_parameter("pq_out", [128, NW], f32, isOutput=True)

    ag_in = nc.dram_tensor("ag_in", [LOCAL, C], f32)
    ag_out = nc.dram_tensor("ag_out", [AUGN, C], f32, addr_space="Shared")

    nchunks = -(-T // KT)

    with TileContext(nc) as tc:
        with (
            tc.tile_pool(name="const", bufs=1) as cpool,
            tc.tile_pool(name="gath", bufs=12) as gpool,
            tc.tile_pool(name="sel", bufs=6) as selpool,
            tc.tile_pool(name="work", bufs=4) as wpool,
            tc.tile_pool(name="scr", bufs=2) as spool,
            tc.tile_pool(name="pagg", bufs=2, space="PSUM") as pagg,
            tc.tile_pool(name="pout", bufs=2, space="PSUM") as pout,
        ):
            # ---- load constants ----
            esrc_sb = cpool.tile([128, T], i32, tag="esrc")
            nc.sync.dma_start(out=esrc_sb[:], in_=esrcT_d[:, :])
            econ_sb = cpool.tile([128, 2 * T + WIN], f32, tag="econ")
            nc.sync.dma_start(out=econ_sb[:], in_=econst_d[:, :])
            edst_sb = econ_sb
            ew_col = T
            iota_sb = econ_sb[:, 2 * T : 2 * T + WIN]
            t01_sb = cpool.tile([2, LOCAL], f32, tag="t01")
            nc.sync.dma_start(out=t01_sb[:], in_=t01_d[:, :])
            w1rep_sb = cpool.tile([128, C], f32, tag="w1rep")
            nc.sync.dma_start(out=w1rep_sb[:], in_=w1rep_d[:, :])
            w2rep_sb = cpool.tile([128, C], f32, tag="w2rep")
            nc.sync.dma_start(out=w2rep_sb[:], in_=w2rep_d[:, :])
            wm_sb = []
            r1_sb = []
            for i in range(LAYERS):
                wm = cpool.tile([C, C], f32, tag=f"wm{i}")
                nc.sync.dma_start(out=wm[:], in_=wmats_d[i, :, :])
                wm_sb.append(wm)
                r1 = cpool.tile([2, C], f32, tag=f"r1{i}")
                nc.sync.dma_start(out=r1[:], in_=rank1_d[i, :, :])
                r1_sb.append(r1)
            pq_sb = cpool.tile([128, NW], f32, tag="pq")

            for layer in range(LAYERS):
                table = x_aug if layer == 0 else ag_out
                for pair in range(NPAIR):
                    ps_agg = pagg.tile([128, 128], f32, tag="agg")
                    for half in range(2):
                        w = 2 * pair + half
                        for j in range(int(tw[w])):
                            t = int(woff[w]) + j
                            sel = selpool.tile([128, WIN], f32, tag="sel")
                            nc.vector.tensor_scalar(
                                out=sel[:],
                                in0=iota_sb,
                                scalar1=econ_sb[:, t : t + 1],
                                scalar2=econ_sb[:, T + t : T + t + 1],
                                op0=mybir.AluOpType.is_equal,
                                op1=mybir.AluOpType.mult,
                            )
                            gt = gpool.tile([128, C], f32, tag="g")
                            nc.gpsimd.indirect_dma_start(
                                out=gt[:],
                                out_offset=None,
                                in_=table[:, :],
                                in_offset=bass.IndirectOffsetOnAxis(
                                    ap=esrc_sb[:, t : t + 1], axis=0
                                ),
                            )
                            nc.tensor.matmul(
                                ps_agg[:, half * WIN : (half + 1) * WIN],
                                lhsT=gt[:],
                                rhs=sel[:],
                                start=(j == 0),
                                stop=(j == int(tw[w]) - 1),
                            )
                    aggT = wpool.tile([128, 128], f32, tag="aggT")
                    nc.vector.tensor_copy(out=aggT[:], in_=ps_agg[:])
                    po = pout.tile([128, C], f32, tag="po")
                    nc.tensor.matmul(
                        po[:], lhsT=aggT[:], rhs=wm_sb[layer][:],
                        start=True, stop=False,
                    )
                    nc.tensor.matmul(
                        po[:],
                        lhsT=t01_sb[:, pair * 128 : (pair + 1) * 128],
                        rhs=r1_sb[layer][:],
                        start=False, stop=True,
                    )
                    xw = wpool.tile([128, C], f32, tag="xw")
                    nc.scalar.activation(
                        xw[:], po[:], mybir.ActivationFunctionType.Relu
                    )
                    rows = slice(pair * 128, (pair + 1) * 128)
                    if layer == 0:
                        nc.sync.dma_start(out=ag_in[rows, :], in_=xw[:])
                    else:
                        nc.sync.dma_start(out=x2_out[rows, :], in_=xw[:])
                        scr = spool.tile([128, C], f32, tag="scr")
                        nc.vector.tensor_tensor(
                            out=scr[:], in0=xw[:], in1=w1rep_sb[:],
                            op=mybir.AluOpType.mult,
                        )
                        nc.vector.tensor_reduce(
                            out=pq_sb[:, pair : pair + 1], in_=scr[:],
                            axis=mybir.AxisListType.X, op=mybir.AluOpType.add,
                        )
                        scr2 = spool.tile([128, C], f32, tag="scr")
                        nc.vector.tensor_tensor(
                            out=scr2[:], in0=xw[:], in1=w2rep_sb[:],
                            op=mybir.AluOpType.mult,
                        )
                        nc.vector.tensor_reduce(
                            out=pq_sb[:, NPAIR + pair : NPAIR + pair + 1],
                            in_=scr2[:],
                            axis=mybir.AxisListType.X, op=mybir.AluOpType.add,
                        )
                if layer == 0:
                    nc.gpsimd.collective_compute(
                        "AllGather",
                        mybir.AluOpType.bypass,
                        replica_groups=[list(range(NCORES))],
                        ins=[ag_in[:, :]],
                        outs=[ag_out[:, :]],
                    )

            # raw pq layout [128, NW]; host reorders
            nc.sync.dma_start(out=pq_out[:, :], in_=pq_sb[:, :])

    nc.compile()
    return nc


def _run_device(st, fw, x):
    from concourse.bass_utils import run_bass_kernel_spmd

    x_aug = np.zeros((AUGN, C), np.float32)
    xv = np.asarray(x, np.float32).reshape(NCORES, NPC, C)
    x_aug.reshape(NCORES, LOCAL, C)[:, :NPC, :] = xv

    T = st["T"]
    econst = np.empty((NCORES, 128, 2 * T + WIN), np.float32)
    econst[:, :, :T] = st["edstT"]
    econst[:, :, T : 2 * T] = st["ewT"]
    econst[:, :, 2 * T :] = np.arange(WIN, dtype=np.float32)

    nc = _build_bass(st["T"], st["tw"], st["woff"])

    in_maps = []
    for c in range(NCORES):
        in_maps.append(
            dict(
                x_aug=x_aug,
                esrcT=st["esrcT"][c],
                econst=econst[c],
                t01=st["t01"][c],
                wmats=fw["wmats"],
                rank1=fw["rank1"],
                w1rep=fw["w1rep"],
                w2rep=fw["w2rep"],
            )
        )
    trace = os.environ.get("KERNEL_TRACE", "0") == "1"
    try:
        res = run_bass_kernel_spmd(
            nc, in_maps, core_ids=list(range(NCORES)), trace=trace
        )
    except Exception:
        if not trace:
            raise
        res = run_bass_kernel_spmd(
            nc, in_maps, core_ids=list(range(NCORES)), trace=False
        )
    if trace:
        global LAST_EXEC_NS, LAST_TRACE
        LAST_EXEC_NS = res.exec_time_ns
        LAST_TRACE = res.instructions_and_trace
    x2 = np.concatenate(
        [res.results[c]["x2_out"][:NPC] for c in range(NCORES)], axis=0
    )
    ps, qs = [], []
    for c in range(NCORES):
        raw = res.results[c]["pq_out"]              # [128, NW]
        ps.append(raw[:, :NPAIR].T.reshape(-1)[:NPC])
        qs.append(raw[:, NPAIR:NW].T.reshape(-1)[:NPC])
    p = np.concatenate(ps)
    q = np.concatenate(qs)
    return x2, p, q


LAST_EXEC_NS = None
LAST_TRACE = None


def _greedy_match(score, src, dst):
    """Exact greedy maximal matching in descending (score, -index) order via
    locally-dominant-edge iteration. Returns (chosen_mask, order_rank_fn)."""
    Em = len(score)
    # Strict total priority: score desc, index asc (matches stable argsort).
    # Scale 2^44 keeps both parts exact in f64 (score is f32 in (0.5, 1.6],
    # so score*2^44 uses bits 2^20..2^45 and the index term < 2^18 sits
    # strictly below the smallest possible nonzero score difference).
    prio = score.astype(np.float64) * (1 << 44) + (Em - np.arange(Em))
    alive = np.ones(Em, bool)
    matched = np.zeros(N, bool)
    chosen = np.zeros(Em, bool)
    idx = np.arange(Em)
    cur = idx
    while len(cur):
        u = src[cur]
        v = dst[cur]
        pr = prio[cur]
        best = np.zeros(N, np.float64)
        np.maximum.at(best, u, pr)
        np.maximum.at(best, v, pr)
        dom = (pr == best[u]) & (pr == best[v])
        ce = cur[dom]
        chosen[ce] = True
        matched[src[ce]] = True
        matched[dst[ce]] = True
        keep = ~(matched[u] | matched[v])
        cur = cur[keep]
    return chosen


def _host_finish(st, fw, x2, p, q, batch):
    src = st["src"]
    dst = st["dst"]
    e = (p[src] + q[dst] + np.float32(fw["k0"])).astype(np.float32)
    m = np.full(N, -np.inf, np.float32)
    np.maximum.at(m, dst, e)
    ee = np.exp(e - m[dst])
    denom = np.zeros(N, np.float32)
    np.add.at(denom, dst, ee)
    score = (ee / denom[dst] + np.float32(ADD_TO_EDGE_SCORE)).astype(np.float32)

    chosen = _greedy_match(score, src, dst)
    ch = np.nonzero(chosen)[0]
    # cluster ids = rank among chosen edges in (score desc, index asc) order
    order = np.argsort(-score[ch], kind="stable")
    ch_sorted = ch[order]
    count = len(ch_sorted)
    cids = np.arange(count, dtype=np.int32)

    cluster = np.full(N, -1, np.int32)
    cluster[src[ch_sorted]] = cids
    cluster[dst[ch_sorted]] = cids
    unmatched = cluster < 0
    cluster = np.where(
        unmatched, count + np.cumsum(unmatched.astype(np.int32)) - 1, cluster
    ).astype(np.int32)

    cscore = np.ones(N, np.float32)
    cscore[cids] = score[ch_sorted]

    # members per cluster (1 or 2 nodes)
    x3 = (x2 * fw["pa"] + fw["pc"]).astype(np.float32)
    nodes_by_cluster = np.argsort(cluster, kind="stable")
    counts = np.bincount(cluster, minlength=N)
    starts = np.concatenate([[0], np.cumsum(counts)[:-1]])
    ncl = count + int(unmatched.sum())
    a_idx = nodes_by_cluster[starts[:ncl]]
    has2 = counts[:ncl] == 2
    b_idx = np.where(
        has2, nodes_by_cluster[np.minimum(starts[:ncl] + 1, N - 1)], a_idx
    )
    new_x = np.zeros((N, C), np.float32)
    sx = x3[a_idx]
    sx[has2] = sx[has2] + x3[b_idx[has2]]
    new_x[:ncl] = sx * cscore[:ncl, None]

    batch = np.asarray(batch)
    nb = batch[a_idx].copy()
    nb[has2] = np.maximum(nb[has2], batch[b_idx[has2]])
    new_batch = np.zeros(N, np.int32)
    new_batch[:ncl] = np.maximum(nb, 0)

    new_edge_index = np.stack([cluster[src], cluster[dst]]).astype(np.int32)
    return new_x, new_edge_index, new_batch


def kernel(**inputs):
    x = np.asarray(inputs["x"], np.float32)
    edge_index = np.asarray(inputs["edge_index"])
    batch = np.asarray(inputs["batch"])
    st = _prep_structure(edge_index)
    fw = _fold_weights({k: np.asarray(v) for k, v in inputs.items()})
    x2, p, q = _run_device(st, fw, x)
    return _host_finish(st, fw, x2, p, q, batch)


# revision 9
# speedup vs baseline: 1.0012x; 1.0012x over previous
"""Trainium2 Bass kernel for nn_ConvPoolBlock (GCN x2 + EdgePooling).

Strategy (8 NeuronCores, SPMD single NEFF):
  - Nodes are sharded contiguously across cores (6250/core, padded to 6272).
  - Edges (incl. GCN self-loops) are assigned to the core owning their dst
    node, sorted by dst, grouped into 64-node windows, padded to 128-edge
    tiles.
  - Per layer: indirect-DMA gather of source-node rows, per-tile one-hot
    selection matrix (built in one DVE op), PE matmul accumulates the
    segment-sum in PSUM per window, then a folded (BN+GCN-norm+bias) weight
    matmul + ReLU produces the layer output window.  BN scale/shift, degree
    normalization and biases are folded into the weights / rank-1 terms on
    the host (structure-only preprocessing).
  - One AllGather between the two conv layers redistributes full x1.
  - The final pool BN is folded into per-node score projections p, q which
    the device computes per window.
  - Host: edge scores e = p[src]+q[dst]+k0, softmax grouped by dst, the
    (inherently sequential) greedy edge matching via the exact
    locally-dominant-edge iteration, cluster building, and the final
    new_x/new_edge_index/new_batch assembly.
"""
import math
import os

import numpy as np

# Problem constants (hardcoded per spec).
N, E, C, NGRAPH = 50000, 250000, 128, 64
LAYERS = 2
EPS = 1e-5
ADD_TO_EDGE_SCORE = 0.5

NCORES = 8
NPC = N // NCORES          # real nodes per core (6250)
WIN = 64                   # nodes per aggregation window
LOCAL = 6272               # padded nodes per core (multiple of 128)
NW = LOCAL // WIN          # windows per core (98)
NPAIR = NW // 2            # window pairs (128-node groups) per core (49)
AUGN = LOCAL * NCORES      # augmented global node count (50176)
TILE = 128                 # edges per tile
KT = 16                    # tiles per gather chunk

_DUMMY = NPC               # augmented-local id of a guaranteed zero row (pad)


def _aug_id(node):
    """Original node id -> augmented (padded) id."""
    c = node // NPC
    return c * LOCAL + (node - c * NPC)


def _prep_structure(edge_index):
    """Host-side graph preprocessing (int/scalar structure only)."""
    src = np.asarray(edge_index[0]).astype(np.int64)
    dst = np.asarray(edge_index[1]).astype(np.int64)
    loop = np.arange(N, dtype=np.int64)
    all_src = np.concatenate([src, loop])
    all_dst = np.concatenate([dst, loop])

    deg = np.bincount(all_dst, minlength=N).astype(np.float64)
    dinv64 = 1.0 / np.sqrt(deg)
    dinv = dinv64.astype(np.float32)
    w_all = (dinv[all_src] * dinv[all_dst]).astype(np.float32)

    # t0[d] = sum of edge weights (incl self loop) into d
    t0 = np.bincount(all_dst, weights=w_all.astype(np.float64), minlength=N)
    t0 = t0.astype(np.float32)

    core = all_dst // NPC
    local = all_dst - core * NPC
    win = local // WIN
    dstloc = (local - win * WIN).astype(np.float32)
    key = (core * NW + win).astype(np.int64)

    order = np.argsort(key, kind="stable")
    key_s = key[order]
    src_s = all_src[order]
    dstloc_s = dstloc[order]
    w_s = w_all[order]

    ngroups = NCORES * NW
    cnt = np.bincount(key_s, minlength=ngroups)          # [ngroups]
    grp_start = np.concatenate([[0], np.cumsum(cnt)[:-1]])
    rank = np.arange(len(key_s)) - grp_start[key_s]      # rank within group

    cnt2 = cnt.reshape(NCORES, NW)
    tiles_cw = -(-cnt2 // TILE)                          # ceil
    tw = tiles_cw.max(axis=0)                            # [NW] max over cores
    tw = np.maximum(tw, 1)
    woff = np.concatenate([[0], np.cumsum(tw)[:-1]])     # tile offset of window
    T = int(tw.sum())

    # padded per-core tile arrays
    esrc = np.full((NCORES, T * TILE), _DUMMY, np.int32)
    edst = np.full((NCORES, T * TILE), -1.0, np.float32)
    ew = np.zeros((NCORES, T * TILE), np.float32)

    cores_s = key_s // NW
    wins_s = key_s - cores_s * NW
    pos = woff[wins_s] * TILE + rank                     # position in stream
    src_aug = (src_s // NPC) * LOCAL + (src_s % NPC)
    esrc[cores_s, pos] = src_aug.astype(np.int32)
    edst[cores_s, pos] = dstloc_s
    ew[cores_s, pos] = w_s

    # transpose to [NCORES, 128, T] (partition-major for SBUF)
    esrcT = np.ascontiguousarray(esrc.reshape(NCORES, T, TILE).transpose(0, 2, 1))
    edstT = np.ascontiguousarray(edst.reshape(NCORES, T, TILE).transpose(0, 2, 1))
    ewT = np.ascontiguousarray(ew.reshape(NCORES, T, TILE).transpose(0, 2, 1))

    # t01 [NCORES, 2, LOCAL]: row0 = t0 (pads 0), row1 = ones
    t01 = np.zeros((NCORES, 2, LOCAL), np.float32)
    t01[:, 1, :] = 1.0
    t01[:, 0, :NPC] = t0.reshape(NCORES, NPC)

    return dict(
        T=T, tw=tw, woff=woff, esrcT=esrcT, edstT=edstT, ewT=ewT,
        t01=t01, dinv=dinv, src=src, dst=dst,
    )


def _fold_weights(inp):
    """Fold BN/bias/pool params into matmul weights (host, f32)."""
    out = {}
    wmats = np.zeros((LAYERS, C, C), np.float32)
    rank1 = np.zeros((LAYERS, 2, C), np.float32)
    for i in range(LAYERS):
        a = (inp["bn_gamma"][i] / np.sqrt(inp["bn_var"][i] + EPS)).astype(np.float32)
        c = (inp["bn_beta"][i] - inp["bn_mean"][i] * a).astype(np.float32)
        wmats[i] = a[:, None] * inp["conv_w"][i]
        rank1[i, 0] = c @ inp["conv_w"][i]
        rank1[i, 1] = inp["conv_b"][i]
    pa = (inp["pool_bn_gamma"] / np.sqrt(inp["pool_bn_var"] + EPS)).astype(np.float32)
    pc = (inp["pool_bn_beta"] - inp["pool_bn_mean"] * pa).astype(np.float32)
    w1 = np.asarray(inp["pool_w"][:C])
    w2 = np.asarray(inp["pool_w"][C:])
    out["wmats"] = wmats
    out["rank1"] = rank1
    out["w1rep"] = np.broadcast_to(pa * w1, (128, C)).copy().astype(np.float32)
    out["w2rep"] = np.broadcast_to(pa * w2, (128, C)).copy().astype(np.float32)
    out["k0"] = float(pc @ (w1 + w2) + np.asarray(inp["pool_b"]))
    out["pa"] = pa
    out["pc"] = pc
    return out


def _build_bass(T, tw, woff):
    """Build the SPMD Bass/Tile program (shared by all 8 cores)."""
    import concourse.bass as bass
    import concourse.bacc as bacc
    import concourse.mybir as mybir
    from concourse.tile import TileContext

    f32 = mybir.dt.float32
    i32 = mybir.dt.int32

    nc = bacc.Bacc()
    x_aug = nc.declare_dram_parameter("x_aug", [AUGN, C], f32, isOutput=False)
    esrcT_d = nc.declare_dram_parameter("esrcT", [128, T], i32, isOutput=False)
    econst_d = nc.declare_dram_parameter("econst", [128, 2 * T + WIN], f32,
                                         isOutput=False)
    t01_d = nc.declare_dram_parameter("t01", [2, LOCAL], f32, isOutput=False)
    wmats_d = nc.declare_dram_parameter("wmats", [LAYERS, C, C], f32, isOutput=False)
    rank1_d = nc.declare_dram_parameter("rank1", [LAYERS, 2, C], f32, isOutput=False)
    w1rep_d = nc.declare_dram_parameter("w1rep", [128, C], f32, isOutput=False)
    w2rep_d = nc.declare_dram_parameter("w2rep", [128, C], f32, isOutput=False)

    x2_out = nc.declare_dram_parameter("x2_out", [LOCAL, C], f32, isOutput=True)
    pq_out = nc.declare_dram_parameter("pq_out", [128, NW], f32, isOutput=True)

    ag_in = nc.dram_tensor("ag_in", [LOCAL, C], f32)
    ag_out = nc.dram_tensor("ag_out", [AUGN, C], f32, addr_space="Shared")

    nchunks = -(-T // KT)

    with TileContext(nc) as tc:
        with (
            tc.tile_pool(name="const", bufs=1) as cpool,
            tc.tile_pool(name="gath", bufs=32) as gpool,
            tc.tile_pool(name="sel", bufs=12) as selpool,
            tc.tile_pool(name="work", bufs=4) as wpool,
            tc.tile_pool(name="scr", bufs=2) as spool,
            tc.tile_pool(name="pagg", bufs=2, space="PSUM") as pagg,
            tc.tile_pool(name="pout", bufs=2, space="PSUM") as pout,
        ):
            # ---- load constants ----
            esrc_sb = cpool.tile([128, T], i32, tag="esrc")
            nc.sync.dma_start(out=esrc_sb[:], in_=esrcT_d[:, :])
            econ_sb = cpool.tile([128, 2 * T + WIN], f32, tag="econ")
            nc.sync.dma_start(out=econ_sb[:], in_=econst_d[:, :])
            edst_sb = econ_sb
            ew_col = T
            iota_sb = econ_sb[:, 2 * T : 2 * T + WIN]
            t01_sb = cpool.tile([2, LOCAL], f32, tag="t01")
            nc.sync.dma_start(out=t01_sb[:], in_=t01_d[:, :])
            w1rep_sb = cpool.tile([128, C], f32, tag="w1rep")
            nc.sync.dma_start(out=w1rep_sb[:], in_=w1rep_d[:, :])
            w2rep_sb = cpool.tile([128, C], f32, tag="w2rep")
            nc.sync.dma_start(out=w2rep_sb[:], in_=w2rep_d[:, :])
            wm_sb = []
            r1_sb = []
            for i in range(LAYERS):
                wm = cpool.tile([C, C], f32, tag=f"wm{i}")
                nc.sync.dma_start(out=wm[:], in_=wmats_d[i, :, :])
                wm_sb.append(wm)
                r1 = cpool.tile([2, C], f32, tag=f"r1{i}")
                nc.sync.dma_start(out=r1[:], in_=rank1_d[i, :, :])
                r1_sb.append(r1)
            pq_sb = cpool.tile([128, NW], f32, tag="pq")

            for layer in range(LAYERS):
                table = x_aug if layer == 0 else ag_out
                for pair in range(NPAIR):
                    ps_agg = pagg.tile([128, 128], f32, tag="agg")
                    for half in range(2):
                        w = 2 * pair + half
                        for j in range(int(tw[w])):
                            t = int(woff[w]) + j
                            sel = selpool.tile([128, WIN], f32, tag="sel")
                            nc.vector.tensor_scalar(
                                out=sel[:],
                                in0=iota_sb,
                                scalar1=econ_sb[:, t : t + 1],
                                scalar2=econ_sb[:, T + t : T + t + 1],
                                op0=mybir.AluOpType.is_equal,
                                op1=mybir.AluOpType.mult,
                            )
                            gt = gpool.tile([128, C], f32, tag="g")
                            nc.gpsimd.indirect_dma_start(
                                out=gt[:],
                                out_offset=None,
                                in_=table[:, :],
                                in_offset=bass.IndirectOffsetOnAxis(
                                    ap=esrc_sb[:, t : t + 1], axis=0
                                ),
                            )
                            nc.tensor.matmul(
                                ps_agg[:, half * WIN : (half + 1) * WIN],
                                lhsT=gt[:],
                                rhs=sel[:],
                                start=(j == 0),
                                stop=(j == int(tw[w]) - 1),
                            )
                    aggT = wpool.tile([128, 128], f32, tag="aggT")
                    nc.vector.tensor_copy(out=aggT[:], in_=ps_agg[:])
                    po = pout.tile([128, C], f32, tag="po")
                    nc.tensor.matmul(
                        po[:], lhsT=aggT[:], rhs=wm_sb[layer][:],
                        start=True, stop=False,
                    )
                    nc.tensor.matmul(
                        po[:],
                        lhsT=t01_sb[:, pair * 128 : (pair + 1) * 128],
                        rhs=r1_sb[layer][:],
                        start=False, stop=True,
                    )
                    xw = wpool.tile([128, C], f32, tag="xw")
                    nc.scalar.activation(
                        xw[:], po[:], mybir.ActivationFunctionType.Relu
                    )
                    rows = slice(pair * 128, (pair + 1) * 128)
                    if layer == 0:
                        nc.sync.dma_start(out=ag_in[rows, :], in_=xw[:])
                    else:
                        nc.sync.dma_start(out=x2_out[rows, :], in_=xw[:])
                        scr = spool.tile([128, C], f32, tag="scr")
                        nc.vector.tensor_tensor(
                            out=scr[:], in0=xw[:], in1=w1rep_sb[:],
                            op=mybir.AluOpType.mult,
                        )
                        nc.vector.tensor_reduce(
                            out=pq_sb[:, pair : pair + 1], in_=scr[:],
                            axis=mybir.AxisListType.X, op=mybir.AluOpType.add,
                        )
                        scr2 = spool.tile([128, C], f32, tag="scr")
                        nc.vector.tensor_tensor(
                            out=scr2[:], in0=xw[:], in1=w2rep_sb[:],
                            op=mybir.AluOpType.mult,
                        )
                        nc.vector.tensor_reduce(
                            out=pq_sb[:, NPAIR + pair : NPAIR + pair + 1],
                            in_=scr2[:],
                            axis=mybir.AxisListType.X, op=mybir.AluOpType.add,
                        )
                if layer == 0:
                    nc.gpsimd.collective_compute(
                        "AllGather",
                        mybir.AluOpType.bypass,
                        replica_groups=[list(range(NCORES))],
                        ins=[ag_in[:, :]],
                        outs=[ag_out[:, :]],
                    )

            # raw pq layout [128, NW]; host reorders
            nc.sync.dma_start(out=pq_out[:, :], in_=pq_sb[:, :])

    nc.compile()
    return nc


def _run_device(st, fw, x):
    from concourse.bass_utils import run_bass_kernel_spmd

    x_aug = np.zeros((AUGN, C), np.float32)
    xv = np.asarray(x, np.float32).reshape(NCORES, NPC, C)
    x_aug.reshape(NCORES, LOCAL, C)[:, :NPC, :] = xv

    T = st["T"]
    econst = np.empty((NCORES, 128, 2 * T + WIN), np.float32)
    econst[:, :, :T] = st["edstT"]
    econst[:, :, T : 2 * T] = st["ewT"]
    econst[:, :, 2 * T :] = np.arange(WIN, dtype=np.float32)

    nc = _build_bass(st["T"], st["tw"], st["woff"])

    in_maps = []
    for c in range(NCORES):
        in_maps.append(
            dict(
                x_aug=x_aug,
                esrcT=st["esrcT"][c],
                econst=econst[c],
                t01=st["t01"][c],
                wmats=fw["wmats"],
                rank1=fw["rank1"],
                w1rep=fw["w1rep"],
                w2rep=fw["w2rep"],
            )
        )
    trace = os.environ.get("KERNEL_TRACE", "0") == "1"
    try:
        res = run_bass_kernel_spmd(
            nc, in_maps, core_ids=list(range(NCORES)), trace=trace
        )
    except Exception:
        if not trace:
            raise
        res = run_bass_kernel_spmd(
            nc, in_maps, core_ids=list(range(NCORES)), trace=False
        )
    if trace:
        global LAST_EXEC_NS, LAST_TRACE
        LAST_EXEC_NS = res.exec_time_ns
        LAST_TRACE = res.instructions_and_trace
    x2 = np.concatenate(
        [res.results[c]["x2_out"][:NPC] for c in range(NCORES)], axis=0
    )
    ps, qs = [], []
    for c in range(NCORES):
        raw = res.results[c]["pq_out"]              # [128, NW]
        ps.append(raw[:, :NPAIR].T.reshape(-1)[:NPC])
        qs.append(raw[:, NPAIR:NW].T.reshape(-1)[:NPC])
    p = np.concatenate(ps)
    q = np.concatenate(qs)
    return x2, p, q


LAST_EXEC_NS = None
LAST_TRACE = None


def _greedy_match(score, src, dst):
    """Exact greedy maximal matching in descending (score, -index) order via
    locally-dominant-edge iteration. Returns (chosen_mask, order_rank_fn)."""
    Em = len(score)
    # Strict total priority: score desc, index asc (matches stable argsort).
    # Scale 2^44 keeps both parts exact in f64 (score is f32 in (0.5, 1.6],
    # so score*2^44 uses bits 2^20..2^45 and the index term < 2^18 sits
    # strictly below the smallest possible nonzero score difference).
    prio = score.astype(np.float64) * (1 << 44) + (Em - np.arange(Em))
    alive = np.ones(Em, bool)
    matched = np.zeros(N, bool)
    chosen = np.zeros(Em, bool)
    idx = np.arange(Em)
    cur = idx
    while len(cur):
        u = src[cur]
        v = dst[cur]
        pr = prio[cur]
        best = np.zeros(N, np.float64)
        np.maximum.at(best, u, pr)
        np.maximum.at(best, v, pr)
        dom = (pr == best[u]) & (pr == best[v])
        ce = cur[dom]
        chosen[ce] = True
        matched[src[ce]] = True
        matched[dst[ce]] = True
        keep = ~(matched[u] | matched[v])
        cur = cur[keep]
    return chosen


def _host_finish(st, fw, x2, p, q, batch):
    src = st["src"]
    dst = st["dst"]
    e = (p[src] + q[dst] + np.float32(fw["k0"])).astype(np.float32)
    m = np.full(N, -np.inf, np.float32)
    np.maximum.at(m, dst, e)
    ee = np.exp(e - m[dst])
    denom = np.zeros(N, np.float32)
    np.add.at(denom, dst, ee)
    score = (ee / denom[dst] + np.float32(ADD_TO_EDGE_SCORE)).astype(np.float32)

    chosen = _greedy_match(score, src, dst)
    ch = np.nonzero(chosen)[0]
    # cluster ids = rank among chosen edges in (score desc, index asc) order
    order = np.argsort(-score[ch], kind="stable")
    ch_sorted = ch[order]
    count = len(ch_sorted)
    cids = np.arange(count, dtype=np.int32)

    cluster = np.full(N, -1, np.int32)
    cluster[src[ch_sorted]] = cids
    cluster[dst[ch_sorted]] = cids
    unmatched = cluster < 0
    cluster = np.where(
        unmatched, count + np.cumsum(unmatched.astype(np.int32)) - 1, cluster
    ).astype(np.int32)

    cscore = np.ones(N, np.float32)
    cscore[cids] = score[ch_sorted]

    # members per cluster (1 or 2 nodes)
    x3 = (x2 * fw["pa"] + fw["pc"]).astype(np.float32)
    nodes_by_cluster = np.argsort(cluster, kind="stable")
    counts = np.bincount(cluster, minlength=N)
    starts = np.concatenate([[0], np.cumsum(counts)[:-1]])
    ncl = count + int(unmatched.sum())
    a_idx = nodes_by_cluster[starts[:ncl]]
    has2 = counts[:ncl] == 2
    b_idx = np.where(
        has2, nodes_by_cluster[np.minimum(starts[:ncl] + 1, N - 1)], a_idx
    )
    new_x = np.zeros((N, C), np.float32)
    sx = x3[a_idx]
    sx[has2] = sx[has2] + x3[b_idx[has2]]
    new_x[:ncl] = sx * cscore[:ncl, None]

    batch = np.asarray(batch)
    nb = batch[a_idx].copy()
    nb[has2] = np.maximum(nb[has2], batch[b_idx[has2]])
    new_batch = np.zeros(N, np.int32)
    new_batch[:ncl] = np.maximum(nb, 0)

    new_edge_index = np.stack([cluster[src], cluster[dst]]).astype(np.int32)
    return new_x, new_edge_index, new_batch


def kernel(**inputs):
    x = np.asarray(inputs["x"], np.float32)
    edge_index = np.asarray(inputs["edge_index"])
    batch = np.asarray(inputs["batch"])
    st = _prep_structure(edge_index)
    fw = _fold_weights({k: np.asarray(v) for k, v in inputs.items()})
    x2, p, q = _run_device(st, fw, x)
    return _host_finish(st, fw, x2, p, q, batch)
